# revision 17
# baseline (speedup 1.0000x reference)
"""GQA forward (B=2,N=2048,D=2048,H=32,KV=8,DH=64, causal) on 8 trn2 cores.

Sharding: 2-way data parallel over batch x 4-way tensor parallel over heads
(each core: 8 q-heads = 2 kv-heads, keeping group structure). Row-parallel
out-proj; the all-reduce over the 4 TP shards (+ bias) happens on host at
gather time.

v2: all matmul inputs bf16 (fp32 PSUM accumulate), kv-head-paired attention
with row-tiled concurrent scores matmuls (K=64 each, disjoint PE row groups),
one [128,1024] 2-bank PSUM scores tile per key block so a single ACT exp
covers both heads, and a per-q-block pipeline (proj(nb+1) / outproj(nb)
matmuls fill the PE while attention waits on ACT) to keep the PE HAM-warm.

Device kernel (per core):
  proj:  streaming projections from xT (host-pretransposed, bf16):
         Q^T (4 pair-slabs), K^T, V^T -> V (PE transpose) into vaug with
         ones columns (softmax row-sums for free in the ctx matmul)
  attn:  per pair (g, kv0/kv1), per 128-key block: two row-tiled scores
         matmuls S^T = K^T.T @ Q^T into one 2-bank PSUM tile, one exp on
         ACT (scale=1/sqrt(dh)), triangle-mask multiply on diagonal
         blocks, ctx^T accumulated per head in PSUM with row 64 = softmax
         denominator; normalize on the PSUM->SBUF copy.
  out:   out = ctx @ Wo_shard accumulated over 4 contraction chunks,
         written bf16; host sums the 4 TP partials + bias in fp32.
"""
import os
import sys

import numpy as np

if "/opt/trn_rl_repo" not in sys.path:
    sys.path.insert(0, "/opt/trn_rl_repo")

import ml_dtypes

import concourse.bacc as bacc
import concourse.tile as tile
from concourse import mybir
from concourse.bass_utils import run_bass_kernel_spmd
from concourse.masks import make_identity

F32 = mybir.dt.float32
F32R = mybir.dt.float32r
BF16 = mybir.dt.bfloat16
EXP = mybir.ActivationFunctionType.Exp

B, N, D = 2, 2048, 2048
H, KV, DH = 32, 8, 64
G = H // KV                      # 4 q-heads per kv head
HPC, KVPC = 8, 2                 # heads / kv-heads per core
DQ = HPC * DH                    # 512 per-core q projection width
NBW = 512                        # q-block width for attention
NB = N // NBW                    # 4 q-blocks
DC = D // 128                    # 16 contraction chunks
NT = N // 128                    # 16 row tiles

_CACHED = {}


def _build():
    nc = bacc.Bacc("TRN2", target_bir_lowering=False, debug=False, num_devices=8)

    xT = nc.dram_tensor("xT", [D, N], BF16, kind="ExternalInput")
    Wq = nc.dram_tensor("Wq", [D, DQ], BF16, kind="ExternalInput")
    Wk = nc.dram_tensor("Wk", [D, KVPC * DH], BF16, kind="ExternalInput")
    Wv = nc.dram_tensor("Wv", [D, KVPC * DH], BF16, kind="ExternalInput")
    Wo = nc.dram_tensor("Wo", [DQ, D], BF16, kind="ExternalInput")
    OUT = nc.dram_tensor("out", [N, D], BF16, kind="ExternalOutput")

    scale = 1.0 / np.sqrt(DH)

    with tile.TileContext(nc) as tc:
        with (
            tc.tile_pool(name="persist", bufs=1) as pp,
            tc.tile_pool(name="wts", bufs=16) as wp,
            tc.tile_pool(name="xs", bufs=16) as xsp,
            tc.tile_pool(name="vt", bufs=2) as vtp,
            tc.tile_pool(name="pt", bufs=4) as ptp,
            tc.tile_pool(name="outs", bufs=2) as outp,
            tc.tile_pool(name="small", bufs=2) as smp,
            tc.tile_pool(name="ps", bufs=1, space="PSUM") as psp,
        ):
            # ---- persistent sbuf state ----
            ident_f = pp.tile([128, 128], F32, tag="identf")
            make_identity(nc, ident_f[:])
            ident = pp.tile([128, 128], BF16, tag="ident")
            nc.vector.tensor_copy(ident[:], ident_f[:])

            # triangle mask: tri[r, j] = 1 if j >= r else 0
            tri_f = pp.tile([128, 128], F32, tag="trif")
            nc.gpsimd.memset(tri_f[:], 1.0)
            nc.gpsimd.affine_select(
                out=tri_f[:], in_=tri_f[:],
                compare_op=mybir.AluOpType.is_ge,
                fill=0.0, base=0,
                pattern=[[1, 128]],
                channel_multiplier=-1,
            )
            tri = pp.tile([128, 128], BF16, tag="tri")
            nc.vector.tensor_copy(tri[:], tri_f[:])

            ones_t = pp.tile([128, DH], BF16, tag="ones")
            nc.vector.memset(ones_t[:], 1.0)

            # q slabs: slab s = [kv0 g=s (rows 0:64) | kv1 g=s (rows 64:128)]
            qt = [pp.tile([128, N], BF16, tag=f"qt{s}", name=f"qt{s}")
                  for s in range(4)]
            kt = pp.tile([128, N], BF16, tag="kt")
            # vaug[m]: [0:64]=v_kv0, 64=ones, [65:129]=v_kv1, 129=ones
            vaug = [pp.tile([128, 2 * (DH + 1)], BF16, tag=f"va{m}",
                            name=f"va{m}") for m in range(NT)]
            for m in range(NT):
                nc.vector.memset(vaug[m][:], 1.0)
            ctxT = [pp.tile([128, N], BF16, tag=f"ct{j}", name=f"ct{j}")
                    for j in range(4)]

            # ---- input DMAs (k/v + first x block first) ----
            wk_sb, wv_sb, wq_sb = [], [], []
            xsf = []
            for dc in range(DC):
                t = wp.tile([128, KVPC * DH], BF16, tag="wk", name="wkt")
                nc.gpsimd.dma_start(out=t[:], in_=Wk[dc * 128:(dc + 1) * 128, :])
                wk_sb.append(t)
                x0 = xsp.tile([128, N], BF16, tag="xs", name="xst")
                nc.sync.dma_start(out=x0[:, 0:NBW],
                                  in_=xT[dc * 128:(dc + 1) * 128, 0:NBW])
                xsf.append(x0)
            for dc in range(DC):
                t = wp.tile([128, KVPC * DH], BF16, tag="wv", name="wvt")
                nc.gpsimd.dma_start(out=t[:], in_=Wv[dc * 128:(dc + 1) * 128, :])
                wv_sb.append(t)
            for dc in range(DC):
                t = wp.tile([128, DQ], BF16, tag="wq", name="wqt")
                nc.gpsimd.dma_start(out=t[:], in_=Wq[dc * 128:(dc + 1) * 128, :])
                wq_sb.append(t)
            for dc in range(DC):
                nc.sync.dma_start(
                    out=xsf[dc][:, NBW:2 * NBW],
                    in_=xT[dc * 128:(dc + 1) * 128, NBW:2 * NBW])
            for dc in range(DC):
                nc.gpsimd.dma_start(
                    out=xsf[dc][:, 2 * NBW:N],
                    in_=xT[dc * 128:(dc + 1) * 128, 2 * NBW:N])
            xs = [[xsf[dc][:, nb * NBW:(nb + 1) * NBW] for nb in range(NB)]
                  for dc in range(DC)]
            wo_sb = {}
            for j in range(4):
                for ob in range(4):
                    t = wp.tile([128, NBW], BF16, tag="wo", name="wot")
                    nc.gpsimd.dma_start(
                        out=t[:],
                        in_=Wo[j * 128:(j + 1) * 128, ob * NBW:(ob + 1) * NBW])
                    wo_sb[(j, ob)] = t

            # ---- projection helpers ----
            def proj_k(nb):
                ncol = slice(nb * NBW, (nb + 1) * NBW)
                k_ps = psp.tile([128, NBW], F32, tag="prps", name="kps")
                for dc in range(DC):
                    nc.tensor.matmul(k_ps[:], wk_sb[dc][:], xs[dc][nb][:],
                                     start=(dc == 0), stop=(dc == DC - 1))
                nc.vector.tensor_copy(kt[:, ncol], k_ps[:])

            def proj_v(nb):
                v_ps = psp.tile([128, NBW], F32, tag="prps", name="vps")
                for dc in range(DC):
                    nc.tensor.matmul(v_ps[:], wv_sb[dc][:], xs[dc][nb][:],
                                     start=(dc == 0), stop=(dc == DC - 1))
                vts = vtp.tile([128, NBW], BF16, tag="vts")
                nc.vector.tensor_copy(vts[:], v_ps[:])
                for i in range(NBW // 128):
                    mt = nb * (NBW // 128) + i
                    tp = psp.tile([128, 128], BF16, tag="scr", name="tps")
                    nc.tensor.transpose(tp[:], vts[:, i * 128:(i + 1) * 128],
                                        ident[:])
                    nc.vector.tensor_copy(vaug[mt][:, 0:DH], tp[:, 0:DH])
                    nc.vector.tensor_copy(vaug[mt][:, DH + 1:2 * DH + 1],
                                          tp[:, DH:2 * DH])

            def proj_q(nb, s):
                ncol = slice(nb * NBW, (nb + 1) * NBW)
                q_ps = psp.tile([128, NBW], F32, tag="prps", name="qps")
                for dc in range(DC):
                    nc.tensor.matmul(q_ps[:],
                                     wq_sb[dc][:, s * 128:(s + 1) * 128],
                                     xs[dc][nb][:],
                                     start=(dc == 0), stop=(dc == DC - 1))
                nc.vector.tensor_copy(qt[s][:, ncol], q_ps[:])

            # ---- attention ----
            def emit_norm(c_ps, j, par, q0):
                # ctx^T rows /= row 64 (ones-col sums). Broadcast the sums
                # from psum partition 64 to 0:64 with a K=1 ones matmul.
                lrow = smp.tile([128, NBW], BF16, tag="lrow", name="lrow")
                nc.vector.tensor_copy(lrow[DH:DH + 1, :], c_ps[DH:DH + 1, :])
                rb_ps = psp.tile([DH, NBW], F32, tag="scr", name="rbps")
                nc.tensor.matmul(rb_ps[:], ones_t[DH:DH + 1, 0:DH],
                                 lrow[DH:DH + 1, :], start=True, stop=True)
                rb = smp.tile([DH, NBW], F32, tag="rb", name="rb")
                nc.vector.reciprocal_approx_fast(out=rb[:], in_=rb_ps[:])
                if par == 0:
                    nc.vector.tensor_mul(ctxT[j][0:DH, q0:q0 + NBW],
                                         c_ps[0:DH, :], rb[:])
                else:
                    tmp = smp.tile([DH, NBW], BF16, tag="ctmp", name="ctmp")
                    nc.vector.tensor_mul(tmp[:], c_ps[0:DH, :], rb[:])
                    nc.sync.dma_start(out=ctxT[j][DH:2 * DH, q0:q0 + NBW],
                                      in_=tmp[:])

            def attn(nb):
                q0 = nb * NBW
                n_mb = 4 * nb + 4
                for s in range(4):
                    j, par = s // 2, s % 2
                    c0 = psp.tile([DH + 1, NBW], F32, tag="cps", name="c0",
                                  bufs=2)
                    c1 = psp.tile([DH + 1, NBW], F32, tag="cps", name="c1",
                                  bufs=2)
                    for mb in range(n_mb):
                        m0 = mb * 128
                        off = max(0, m0 - q0)
                        w = NBW - off
                        sp = psp.tile([128, 2 * NBW], F32, tag="sps",
                                      name="sps", bufs=2)
                        nc.tensor.matmul(
                            sp[:, 0:w],
                            kt[0:DH, m0:m0 + 128],
                            qt[s][0:DH, q0 + off:q0 + NBW],
                            start=True, stop=True)
                        nc.tensor.matmul(
                            sp[:, NBW:NBW + w],
                            kt[DH:128, m0:m0 + 128],
                            qt[s][DH:128, q0 + off:q0 + NBW],
                            start=True, stop=True)
                        p = ptp.tile([128, 2 * NBW], BF16, tag="pt", name="pt")
                        nc.scalar.activation(p[:, 0:NBW + w], sp[:, 0:NBW + w],
                                             EXP, scale=float(scale))
                        if mb >= 4 * nb:  # diagonal block
                            nc.vector.tensor_mul(p[:, 0:128], p[:, 0:128],
                                                 tri[:])
                            nc.vector.tensor_mul(p[:, NBW:NBW + 128],
                                                 p[:, NBW:NBW + 128], tri[:])
                        st, sp_ = (mb == 0), (mb == n_mb - 1)
                        nc.tensor.matmul(c0[:, off:NBW],
                                         vaug[mb][:, 0:DH + 1],
                                         p[:, 0:w], start=st, stop=sp_)
                        nc.tensor.matmul(c1[:, off:NBW],
                                         vaug[mb][:, DH + 1:2 * (DH + 1)],
                                         p[:, NBW:NBW + w], start=st, stop=sp_)
                    emit_norm(c0, j, par, q0)
                    emit_norm(c1, 2 + j, par, q0)

            # ---- out projection ----
            def outproj(nb):
                # the last block runs after attention: reuse its freed psum
                ptag = "cps" if nb == NB - 1 else "scr"
                pbufs = 2 if nb == NB - 1 else 1
                for nt in range(4 * nb, 4 * nb + 4):
                    o_sb = outp.tile([128, D], BF16, tag="osb", name="osb")
                    for ob in range(4):
                        o_ps = psp.tile([128, NBW], F32, tag=ptag, name="ops",
                                        bufs=pbufs)
                        for j in range(4):
                            nc.tensor.matmul(
                                o_ps[:],
                                ctxT[j][:, nt * 128:(nt + 1) * 128],
                                wo_sb[(j, ob)][:],
                                start=(j == 0), stop=(j == 3))
                        if nb == NB - 1:
                            nc.scalar.copy(o_sb[:, ob * NBW:(ob + 1) * NBW],
                                           o_ps[:])
                        else:
                            nc.vector.tensor_copy(
                                o_sb[:, ob * NBW:(ob + 1) * NBW], o_ps[:])
                    nc.sync.dma_start(out=OUT[nt * 128:(nt + 1) * 128, :],
                                      in_=o_sb[:])

            # ---- program: per-q-block pipeline ----
            # proj(nb+1) sits after attn(nb) in program order (= lower
            # scheduler priority) so it fills PE idle while ACT churns.
            proj_k(0)
            proj_v(0)
            for s in range(4):
                proj_q(0, s)
            for nb in range(NB):
                attn(nb)
                if nb + 1 < NB:
                    proj_k(nb + 1)
                    proj_v(nb + 1)
                    for s in range(4):
                        proj_q(nb + 1, s)
            # out-projections last in program order = lowest priority: the
            # scheduler pulls them forward as PE filler wherever attention
            # is ACT-bound (mostly during attn(3)), leaving a dense tail.
            for nb in range(NB):
                outproj(nb)

    nc.compile()
    return nc


def kernel(x, Wq, Wk, Wv, Wo, bo):
    x = np.asarray(x, dtype=np.float32)
    Wq = np.asarray(Wq, dtype=np.float32)
    Wk = np.asarray(Wk, dtype=np.float32)
    Wv = np.asarray(Wv, dtype=np.float32)
    Wo = np.asarray(Wo, dtype=np.float32)
    bo = np.asarray(bo, dtype=np.float32)
    bf = ml_dtypes.bfloat16

    if "nc" not in _CACHED:
        _CACHED["nc"] = _build()
    nc = _CACHED["nc"]

    in_maps = []
    for c in range(8):
        b, t = c // 4, c % 4
        xT = np.ascontiguousarray(x[b].T).astype(bf)
        # q slab s holds [kv-head 2t head g=s | kv-head 2t+1 head g=s]
        qcols = []
        for s in range(4):
            for kvl in range(KVPC):
                h = (2 * t + kvl) * G + s
                qcols.append(Wq[:, h * DH:(h + 1) * DH])
        wq_c = np.ascontiguousarray(np.concatenate(qcols, axis=1)).astype(bf)
        wk_c = np.ascontiguousarray(Wk[:, t * 128:(t + 1) * 128]).astype(bf)
        wv_c = np.ascontiguousarray(Wv[:, t * 128:(t + 1) * 128]).astype(bf)
        wo_c = np.ascontiguousarray(Wo[t * DQ:(t + 1) * DQ, :]).astype(bf)
        in_maps.append({"xT": xT, "Wq": wq_c, "Wk": wk_c, "Wv": wv_c,
                        "Wo": wo_c})

    trace = bool(int(os.environ.get("GQA_TRACE", "0")))
    kwargs = {}
    if trace:
        import tempfile
        td = os.environ.get("GQA_TRACE_DIR") or tempfile.mkdtemp(prefix="gqa_")
        kwargs = dict(trace=True, tmpdir=td)
    res = run_bass_kernel_spmd(nc, in_maps, list(range(8)), **kwargs)
    _CACHED["last_result"] = res

    out = np.empty((B, N, D), dtype=np.float32)
    for b in range(B):
        acc = res.results[4 * b]["out"].astype(np.float32)
        for t in range(1, 4):
            acc = acc + res.results[4 * b + t]["out"].astype(np.float32)
        out[b] = acc + bo[None, :]
    return out


# revision 19
# speedup vs baseline: 1.1092x; 1.1092x over previous
"""GQA forward (B=2,N=2048,D=2048,H=32,KV=8,DH=64, causal) on 8 trn2 cores.

Sharding: 2-way data parallel over batch x 4-way tensor parallel over heads
(each core: 8 q-heads = 2 kv-heads, keeping group structure). Row-parallel
out-proj; the all-reduce over the 4 TP shards (+ bias) happens on host at
gather time.

v2: all matmul inputs bf16 (fp32 PSUM accumulate), kv-head-paired attention
with row-tiled concurrent scores matmuls (K=64 each, disjoint PE row groups),
one [128,1024] 2-bank PSUM scores tile per key block so a single ACT exp
covers both heads, and a per-q-block pipeline (proj(nb+1) / outproj(nb)
matmuls fill the PE while attention waits on ACT) to keep the PE HAM-warm.

Device kernel (per core):
  proj:  streaming projections from xT (host-pretransposed, bf16):
         Q^T (4 pair-slabs), K^T, V^T -> V (PE transpose) into vaug with
         ones columns (softmax row-sums for free in the ctx matmul)
  attn:  per pair (g, kv0/kv1), per 128-key block: two row-tiled scores
         matmuls S^T = K^T.T @ Q^T into one 2-bank PSUM tile, one exp on
         ACT (scale=1/sqrt(dh)), triangle-mask multiply on diagonal
         blocks, ctx^T accumulated per head in PSUM with row 64 = softmax
         denominator; normalize on the PSUM->SBUF copy.
  out:   out = ctx @ Wo_shard accumulated over 4 contraction chunks,
         written bf16; host sums the 4 TP partials + bias in fp32.
"""
import os
import sys

import numpy as np

if "/opt/trn_rl_repo" not in sys.path:
    sys.path.insert(0, "/opt/trn_rl_repo")

import ml_dtypes

import concourse.bacc as bacc
import concourse.tile as tile
from concourse import mybir
from concourse.bass_utils import run_bass_kernel_spmd
from concourse.masks import make_identity

F32 = mybir.dt.float32
F32R = mybir.dt.float32r
BF16 = mybir.dt.bfloat16
EXP = mybir.ActivationFunctionType.Exp

B, N, D = 2, 2048, 2048
H, KV, DH = 32, 8, 64
G = H // KV                      # 4 q-heads per kv head
HPC, KVPC = 8, 2                 # heads / kv-heads per core
DQ = HPC * DH                    # 512 per-core q projection width
NBW = 512                        # q-block width for attention
NB = N // NBW                    # 4 q-blocks
DC = D // 128                    # 16 contraction chunks
NT = N // 128                    # 16 row tiles

_CACHED = {}


def _build():
    nc = bacc.Bacc("TRN2", target_bir_lowering=False, debug=False, num_devices=8)

    xT = nc.dram_tensor("xT", [D, N], BF16, kind="ExternalInput")
    Wq = nc.dram_tensor("Wq", [D, DQ], BF16, kind="ExternalInput")
    Wk = nc.dram_tensor("Wk", [D, KVPC * DH], BF16, kind="ExternalInput")
    Wv = nc.dram_tensor("Wv", [D, KVPC * DH], BF16, kind="ExternalInput")
    Wo = nc.dram_tensor("Wo", [DQ, D], BF16, kind="ExternalInput")
    OUT = nc.dram_tensor("out", [N, D], BF16, kind="ExternalOutput")

    scale = 1.0 / np.sqrt(DH)

    with tile.TileContext(nc) as tc:
        with (
            tc.tile_pool(name="persist", bufs=1) as pp,
            tc.tile_pool(name="wts", bufs=16) as wp,
            tc.tile_pool(name="xs", bufs=16) as xsp,
            tc.tile_pool(name="vt", bufs=2) as vtp,
            tc.tile_pool(name="pt", bufs=4) as ptp,
            tc.tile_pool(name="outs", bufs=2) as outp,
            tc.tile_pool(name="small", bufs=2) as smp,
            tc.tile_pool(name="ps", bufs=1, space="PSUM") as psp,
        ):
            # ---- persistent sbuf state ----
            ident_f = pp.tile([128, 128], F32, tag="identf")
            make_identity(nc, ident_f[:])
            ident = pp.tile([128, 128], BF16, tag="ident")
            nc.vector.tensor_copy(ident[:], ident_f[:])

            # triangle mask: tri[r, j] = 1 if j >= r else 0
            tri_f = pp.tile([128, 128], F32, tag="trif")
            nc.gpsimd.memset(tri_f[:], 1.0)
            nc.gpsimd.affine_select(
                out=tri_f[:], in_=tri_f[:],
                compare_op=mybir.AluOpType.is_ge,
                fill=0.0, base=0,
                pattern=[[1, 128]],
                channel_multiplier=-1,
            )
            tri = pp.tile([128, 128], BF16, tag="tri")
            nc.vector.tensor_copy(tri[:], tri_f[:])

            ones_t = pp.tile([128, DH], BF16, tag="ones")
            nc.vector.memset(ones_t[:], 1.0)

            # q slabs: slab s = [kv0 g=s (rows 0:64) | kv1 g=s (rows 64:128)]
            qt = [pp.tile([128, N], BF16, tag=f"qt{s}", name=f"qt{s}")
                  for s in range(4)]
            kt = pp.tile([128, N], BF16, tag="kt")
            # vaug[m]: [0:64]=v_kv0, 64=ones, [65:129]=v_kv1, 129=ones
            vaug = [pp.tile([128, 2 * (DH + 1)], BF16, tag=f"va{m}",
                            name=f"va{m}") for m in range(NT)]
            for m in range(NT):
                nc.vector.memset(vaug[m][:], 1.0)
            ctxT = [pp.tile([128, N], BF16, tag=f"ct{j}", name=f"ct{j}")
                    for j in range(4)]

            # ---- input DMAs (k/v + first x block first) ----
            wk_sb, wv_sb, wq_sb = [], [], []
            xsf = []
            for dc in range(DC):
                t = wp.tile([128, KVPC * DH], BF16, tag="wk", name="wkt")
                nc.gpsimd.dma_start(out=t[:], in_=Wk[dc * 128:(dc + 1) * 128, :])
                wk_sb.append(t)
                x0 = xsp.tile([128, N], BF16, tag="xs", name="xst")
                nc.sync.dma_start(out=x0[:, 0:NBW],
                                  in_=xT[dc * 128:(dc + 1) * 128, 0:NBW])
                xsf.append(x0)
            for dc in range(DC):
                t = wp.tile([128, KVPC * DH], BF16, tag="wv", name="wvt")
                nc.gpsimd.dma_start(out=t[:], in_=Wv[dc * 128:(dc + 1) * 128, :])
                wv_sb.append(t)
            for dc in range(DC):
                t = wp.tile([128, DQ], BF16, tag="wq", name="wqt")
                nc.gpsimd.dma_start(out=t[:], in_=Wq[dc * 128:(dc + 1) * 128, :])
                wq_sb.append(t)
            for dc in range(DC):
                nc.sync.dma_start(
                    out=xsf[dc][:, NBW:2 * NBW],
                    in_=xT[dc * 128:(dc + 1) * 128, NBW:2 * NBW])
            for dc in range(DC):
                nc.gpsimd.dma_start(
                    out=xsf[dc][:, 2 * NBW:N],
                    in_=xT[dc * 128:(dc + 1) * 128, 2 * NBW:N])
            xs = [[xsf[dc][:, nb * NBW:(nb + 1) * NBW] for nb in range(NB)]
                  for dc in range(DC)]
            wo_sb = {}
            for j in range(4):
                for ob in range(4):
                    t = wp.tile([128, NBW], BF16, tag="wo", name="wot")
                    nc.gpsimd.dma_start(
                        out=t[:],
                        in_=Wo[j * 128:(j + 1) * 128, ob * NBW:(ob + 1) * NBW])
                    wo_sb[(j, ob)] = t

            # ---- projection helpers ----
            def proj_k(nb):
                ncol = slice(nb * NBW, (nb + 1) * NBW)
                k_ps = psp.tile([128, NBW], F32, tag="prps", name="kps")
                for dc in range(DC):
                    nc.tensor.matmul(k_ps[:], wk_sb[dc][:], xs[dc][nb][:],
                                     start=(dc == 0), stop=(dc == DC - 1))
                nc.vector.tensor_copy(kt[:, ncol], k_ps[:])

            def proj_v(nb):
                v_ps = psp.tile([128, NBW], F32, tag="prps", name="vps")
                for dc in range(DC):
                    nc.tensor.matmul(v_ps[:], wv_sb[dc][:], xs[dc][nb][:],
                                     start=(dc == 0), stop=(dc == DC - 1))
                vts = vtp.tile([128, NBW], BF16, tag="vts")
                nc.vector.tensor_copy(vts[:], v_ps[:])
                for i in range(NBW // 128):
                    mt = nb * (NBW // 128) + i
                    tp = psp.tile([128, 128], BF16, tag="scr", name="tps")
                    nc.tensor.transpose(tp[:], vts[:, i * 128:(i + 1) * 128],
                                        ident[:])
                    nc.vector.tensor_copy(vaug[mt][:, 0:DH], tp[:, 0:DH])
                    nc.vector.tensor_copy(vaug[mt][:, DH + 1:2 * DH + 1],
                                          tp[:, DH:2 * DH])

            def proj_q(nb, s):
                ncol = slice(nb * NBW, (nb + 1) * NBW)
                q_ps = psp.tile([128, NBW], F32, tag="prps", name="qps")
                for dc in range(DC):
                    nc.tensor.matmul(q_ps[:],
                                     wq_sb[dc][:, s * 128:(s + 1) * 128],
                                     xs[dc][nb][:],
                                     start=(dc == 0), stop=(dc == DC - 1))
                nc.vector.tensor_copy(qt[s][:, ncol], q_ps[:])

            # ---- attention ----
            def emit_norm(c_ps, j, par, q0):
                # ctx^T rows /= row 64 (ones-col sums). Broadcast the sums
                # from psum partition 64 to 0:64 with a K=1 ones matmul.
                lrow = smp.tile([128, NBW], BF16, tag="lrow", name="lrow")
                nc.vector.tensor_copy(lrow[DH:DH + 1, :], c_ps[DH:DH + 1, :])
                rb_ps = psp.tile([DH, NBW], F32, tag="scr", name="rbps")
                nc.tensor.matmul(rb_ps[:], ones_t[DH:DH + 1, 0:DH],
                                 lrow[DH:DH + 1, :], start=True, stop=True)
                rb = smp.tile([DH, NBW], F32, tag="rb", name="rb")
                nc.vector.reciprocal_approx_fast(out=rb[:], in_=rb_ps[:])
                if par == 0:
                    nc.vector.tensor_mul(ctxT[j][0:DH, q0:q0 + NBW],
                                         c_ps[0:DH, :], rb[:])
                else:
                    tmp = smp.tile([DH, NBW], BF16, tag="ctmp", name="ctmp")
                    nc.vector.tensor_mul(tmp[:], c_ps[0:DH, :], rb[:])
                    nc.sync.dma_start(out=ctxT[j][DH:2 * DH, q0:q0 + NBW],
                                      in_=tmp[:])

            def attn(nb):
                q0 = nb * NBW
                n_mb = 4 * nb + 4
                for s in range(4):
                    j, par = s // 2, s % 2
                    c0 = psp.tile([DH + 1, NBW], F32, tag="cps", name="c0",
                                  bufs=2)
                    c1 = psp.tile([DH + 1, NBW], F32, tag="cps", name="c1",
                                  bufs=2)
                    for mb in range(n_mb):
                        m0 = mb * 128
                        off = max(0, m0 - q0)
                        w = NBW - off
                        sp = psp.tile([128, 2 * NBW], F32, tag="sps",
                                      name="sps", bufs=2)
                        nc.tensor.matmul(
                            sp[:, 0:w],
                            kt[0:DH, m0:m0 + 128],
                            qt[s][0:DH, q0 + off:q0 + NBW],
                            start=True, stop=True)
                        nc.tensor.matmul(
                            sp[:, NBW:NBW + w],
                            kt[DH:128, m0:m0 + 128],
                            qt[s][DH:128, q0 + off:q0 + NBW],
                            start=True, stop=True)
                        p = ptp.tile([128, 2 * NBW], BF16, tag="pt", name="pt")
                        nc.scalar.activation(p[:, 0:NBW + w], sp[:, 0:NBW + w],
                                             EXP, scale=float(scale))
                        if mb >= 4 * nb:  # diagonal block
                            nc.vector.tensor_mul(p[:, 0:128], p[:, 0:128],
                                                 tri[:])
                            nc.vector.tensor_mul(p[:, NBW:NBW + 128],
                                                 p[:, NBW:NBW + 128], tri[:])
                        st, sp_ = (mb == 0), (mb == n_mb - 1)
                        nc.tensor.matmul(c0[:, off:NBW],
                                         vaug[mb][:, 0:DH + 1],
                                         p[:, 0:w], start=st, stop=sp_)
                        nc.tensor.matmul(c1[:, off:NBW],
                                         vaug[mb][:, DH + 1:2 * (DH + 1)],
                                         p[:, NBW:NBW + w], start=st, stop=sp_)
                    emit_norm(c0, j, par, q0)
                    emit_norm(c1, 2 + j, par, q0)

            # ---- out projection ----
            def outproj(nb):
                # the last block runs after attention: reuse its freed psum.
                # nb=1 borrows the proj accumulator bank (idle by then) so
                # two filler units can be in flight at once.
                ptag = {0: "scr", 1: "prps", 2: "scr", 3: "cps"}[nb]
                pbufs = 2 if nb == NB - 1 else 1
                for nt in range(4 * nb, 4 * nb + 4):
                    o_sb = outp.tile([128, D], BF16, tag="osb", name="osb")
                    for ob in range(4):
                        o_ps = psp.tile([128, NBW], F32, tag=ptag, name="ops",
                                        bufs=pbufs)
                        for j in range(4):
                            nc.tensor.matmul(
                                o_ps[:],
                                ctxT[j][:, nt * 128:(nt + 1) * 128],
                                wo_sb[(j, ob)][:],
                                start=(j == 0), stop=(j == 3))
                        if nb == NB - 1:
                            nc.scalar.copy(o_sb[:, ob * NBW:(ob + 1) * NBW],
                                           o_ps[:])
                        else:
                            nc.vector.tensor_copy(
                                o_sb[:, ob * NBW:(ob + 1) * NBW], o_ps[:])
                    nc.sync.dma_start(out=OUT[nt * 128:(nt + 1) * 128, :],
                                      in_=o_sb[:])

            # ---- program: per-q-block pipeline ----
            # proj(nb+1) sits after attn(nb) in program order (= lower
            # scheduler priority) so it fills PE idle while ACT churns.
            proj_k(0)
            proj_v(0)
            for s in range(4):
                proj_q(0, s)
            # out-projection blocks are placed one attention window late
            # (outproj(nb) between attn(nb+1) and attn(nb+2)) so they act
            # as PE filler during the later, more ACT-bound windows.
            for nb in range(NB):
                attn(nb)
                if nb + 1 < NB:
                    proj_k(nb + 1)
                    proj_v(nb + 1)
                    for s in range(4):
                        proj_q(nb + 1, s)
                if nb >= 1:
                    outproj(nb - 1)
            outproj(NB - 1)

    nc.compile()
    return nc


def kernel(x, Wq, Wk, Wv, Wo, bo):
    x = np.asarray(x, dtype=np.float32)
    Wq = np.asarray(Wq, dtype=np.float32)
    Wk = np.asarray(Wk, dtype=np.float32)
    Wv = np.asarray(Wv, dtype=np.float32)
    Wo = np.asarray(Wo, dtype=np.float32)
    bo = np.asarray(bo, dtype=np.float32)
    bf = ml_dtypes.bfloat16

    if "nc" not in _CACHED:
        _CACHED["nc"] = _build()
    nc = _CACHED["nc"]

    in_maps = []
    for c in range(8):
        b, t = c // 4, c % 4
        xT = np.ascontiguousarray(x[b].T).astype(bf)
        # q slab s holds [kv-head 2t head g=s | kv-head 2t+1 head g=s]
        qcols = []
        for s in range(4):
            for kvl in range(KVPC):
                h = (2 * t + kvl) * G + s
                qcols.append(Wq[:, h * DH:(h + 1) * DH])
        wq_c = np.ascontiguousarray(np.concatenate(qcols, axis=1)).astype(bf)
        wk_c = np.ascontiguousarray(Wk[:, t * 128:(t + 1) * 128]).astype(bf)
        wv_c = np.ascontiguousarray(Wv[:, t * 128:(t + 1) * 128]).astype(bf)
        wo_c = np.ascontiguousarray(Wo[t * DQ:(t + 1) * DQ, :]).astype(bf)
        in_maps.append({"xT": xT, "Wq": wq_c, "Wk": wk_c, "Wv": wv_c,
                        "Wo": wo_c})

    trace = bool(int(os.environ.get("GQA_TRACE", "0")))
    kwargs = {}
    if trace:
        import tempfile
        td = os.environ.get("GQA_TRACE_DIR") or tempfile.mkdtemp(prefix="gqa_")
        kwargs = dict(trace=True, tmpdir=td)
    res = run_bass_kernel_spmd(nc, in_maps, list(range(8)), **kwargs)
    _CACHED["last_result"] = res

    out = np.empty((B, N, D), dtype=np.float32)
    for b in range(B):
        acc = res.results[4 * b]["out"].astype(np.float32)
        for t in range(1, 4):
            acc = acc + res.results[4 * b + t]["out"].astype(np.float32)
        out[b] = acc + bo[None, :]
    return out
